# revision 29
# baseline (speedup 1.0000x reference)
"""Trainium2 Bass kernel for nn_Attention_81836306858184.

Sharding: data-parallel over batch — core b computes batch b
(8 cores, 8 batches, no collectives).

Math: the reference's per-instance softmax over (C*HW) has logits
  L[c,hw] = masks[i,hw] * Wm[i,c] + bm[i,c]
with |Wm * masks| <= ~0.08. A Taylor expansion of exp around the bias
term collapses the 134M-element softmax to a rank factorization; at
K=1 the per-instance softmax sum msum[c] is constant over hw and the
whole module becomes an affine map
  out = M @ x + v + x,
  M = gamma * (Wo * alpha) @ Wf,   alpha[c] = sum_i exp(bm[i,c]) / Z_i,
  v = gamma * ((Wo * alpha) @ bf + I*bo),   Z_i = HW * sum_c exp(bm[i,c])
(measured end-to-end truncation error 2.6e-8 on the reference input
distribution). On that distribution alpha ~ 1.5e-5, so M's entries are
~1e-7 and the M @ x term contributes 7.7e-7 relative — dropping it
leaves out = x + v, verified 7.7e-7 end-to-end. The fast path therefore
streams x through a single DVE broadcast-add of the per-channel column
v, with bf16 HBM staging on both sides (in+out rel err 2.3e-3 vs the
2e-2 gate; halves HBM traffic, which is the roofline for this kernel:
~2.1 MB in + ~2.1 MB out per core at ~358 GB/s).

Host-side guards pick the path per call:
  ||M||_F small + small logits -> fast bf16 x+v path
  small logits only            -> exact f32 collapsed matmul graph
  otherwise                    -> K=2 rank-factorized softmax graph

Scheduling of the fast path: x pieces stream on the two HWDGE rings
(sync: c-tile 0, scalar: v then c-tile 1) so both progress while DVE
adds chase the landed pieces; outputs drain on the gpsimd (SWDGE) ring
so they never queue behind input pieces.
"""
import os
import sys

for _p in ('/opt/trn_rl_repo', '/root/.axon_site/_ro/trn_rl_repo'):
    if os.path.isdir(_p) and _p not in sys.path:
        sys.path.insert(0, _p)

import math
import numpy as np
import ml_dtypes

import concourse.bass as bass
import concourse.tile as tile
from concourse import bacc, mybir
from concourse.bass_utils import run_bass_kernel_spmd

B, I, C, H, W = 8, 16, 256, 64, 64
HW = H * W            # 4096
K = 2                 # Taylor terms (k = 0..K-1) for the fallback graph
IK = I * K            # 32 contraction rows for the msum matmul
NCH = 512             # matmul moving-dim chunk (one PSUM bank)
NHW = HW // NCH       # 8 hw chunks
CT = C // 128         # 2 c-tiles
XQ = 8                # x DMA pieces per c-tile (512KB each)
XQW = HW // XQ        # 512
N_WARM = 8            # dummy matmuls to lift the PE HAM clock gate early
N_PRE = 4             # feat slices emitted before the Z chain

PIECE = 2048          # fast path: columns per DMA/DVE piece (512KB, 4KB/partition)
NP = HW // PIECE      # 2 pieces per c-tile

dt = mybir.dt
AF = mybir.ActivationFunctionType
ALU = mybir.AluOpType
BF16 = ml_dtypes.bfloat16

_nc_cache: dict = {}


def _build_fast():
    """out = bf16(x + v): DMA-in bf16 -> DVE broadcast-add -> DMA-out bf16.

    The input is packed [C, HW+8] with v (bf16) replicated in the trailing
    8 columns of each row, so no separate v DMA exists (a [128, small] DMA's
    128 tiny descriptors trickle through the SDMA round-robin for ~5us when
    the read stream is active, gating the first DVE).

    Reads stream on the sync HWDGE ring, writes on the scalar HWDGE ring,
    chased by the DVE adds. Writes preempt reads at the SDMA engines
    (posted HBM writes drain fast; HBM reads are latency-bound round
    trips), which stretches the read tail — but the overlap still beats
    strict phasing, which pays the full ~1.2us DMA completion receipt at
    the phase boundary plus the entire write stream serially (measured:
    overlap 24.1us vs phased 27.2us end-to-end). Single-ring streams:
    a second ring does not ramp faster, and SWDGE costs ~1.5us Q7
    emission per DMA.
    """
    nc = bacc.Bacc("TRN2", target_bir_lowering=False, debug=False)

    bf = dt.bfloat16
    HWP = HW + 8
    x_d = nc.dram_tensor("xb", [C, HWP], bf, kind="ExternalInput")
    out_d = nc.dram_tensor("out", [C, HW], bf, kind="ExternalOutput")

    # pieces: (ct, lo, hi) into the packed columns; the first pieces carry
    # the v tail so every DVE's scalar operand is resident first.
    rpieces = []
    for ct in range(CT):
        rpieces.append((ct, HW - PIECE, HWP))
    for ct in range(CT):
        for q in range(NP - 2, -1, -1):
            rpieces.append((ct, q * PIECE, (q + 1) * PIECE))

    with tile.TileContext(nc) as tc:
        with (
            tc.tile_pool(name="const", bufs=1) as cpool,
            tc.tile_pool(name="xp", bufs=1) as xpool,
            tc.tile_pool(name="fin", bufs=CT * NP) as opool,
        ):
            x_t = [xpool.tile([128, HWP], bf, tag=f"x{ct}", name=f"x{ct}")
                   for ct in range(CT)]
            for ct, lo, hi in rpieces:
                nc.sync.dma_start(x_t[ct][:, lo:hi],
                                  x_d[ct * 128:(ct + 1) * 128, lo:hi])

            def vbcast(ct, width):
                # bf16(v) = high half of the packed f32 bits, broadcast
                # across the piece via a stride-0 AP: pure-bf16
                # tensor_tensor runs the DVE 2x path (~534ns/piece vs
                # ~900ns for tensor_scalar with an f32 pointer operand).
                ap = x_t[ct][:, HW + 1:HW + 2]
                return bass.AP(ap.tensor, ap.offset, [ap.ap[0], (0, width)])

            for ct, lo, hi in rpieces:
                hi = min(hi, HW)
                fin = opool.tile([128, PIECE], bf, tag="fin",
                                 name=f"fin{ct}_{lo}")
                nc.vector.tensor_add(fin[:, 0:hi - lo], x_t[ct][:, lo:hi],
                                     vbcast(ct, hi - lo))
                nc.scalar.dma_start(out_d[ct * 128:(ct + 1) * 128, lo:hi],
                                    fin[:, 0:hi - lo])

    nc.compile()
    return nc


def _fast_consts(Wf, bf, Wm, bm, Wo, bo, gamma):
    """Collapsed affine map (f64 on host): M, v with out = M@x + v + x."""
    gamma = float(np.asarray(gamma))
    Wf64 = np.asarray(Wf, dtype=np.float64)
    Wo64 = np.asarray(Wo, dtype=np.float64)
    bf64 = np.asarray(bf, dtype=np.float64)
    bo64 = np.asarray(bo, dtype=np.float64)
    E = np.exp(np.asarray(bm, dtype=np.float64))
    Zi = HW * E.sum(axis=1)
    alpha = (E / Zi[:, None]).sum(axis=0)          # [C]
    Woa = Wo64 * alpha[None, :]
    M = gamma * (Woa @ Wf64)                       # [C, C]
    v = (gamma * (Woa @ bf64 + I * bo64)).astype(np.float32)
    return M, v


def _build(gamma: float):
    """K=2 rank-factorized softmax graph (fallback for large logits)."""
    nc = bacc.Bacc("TRN2", target_bir_lowering=False, debug=False)

    f32, f32r = dt.float32, dt.float32r
    x_d = nc.dram_tensor("x", [C, HW], f32r, kind="ExternalInput")
    # pmat rows: 0:16 ones, 16:32 masks  (the K=2 "powers" matrix)
    p_d = nc.dram_tensor("pmat", [IK, HW], f32r, kind="ExternalInput")
    # wf_sb[p, cc*C + o] = Wf[o, cc*128+p] ; same layout for wo_sb
    wf_d = nc.dram_tensor("wf_sb", [128, CT * C], f32r, kind="ExternalInput")
    wo_d = nc.dram_tensor("wo_sb", [128, CT * C], f32r, kind="ExternalInput")
    bf_d = nc.dram_tensor("bf_col", [128, CT], f32, kind="ExternalInput")
    # gamma * I * bo, column layout [128, CT]
    bo_d = nc.dram_tensor("bo_col", [128, CT], f32, kind="ExternalInput")
    t_d = nc.dram_tensor("t_mat", [IK, C], f32, kind="ExternalInput")
    r_d = nc.dram_tensor("r_col", [IK, 1], f32, kind="ExternalInput")
    sel_d = nc.dram_tensor("sel", [IK, I], f32, kind="ExternalInput")
    sel2_d = nc.dram_tensor("sel2", [I, IK], f32, kind="ExternalInput")

    out_d = nc.dram_tensor("out", [C, HW], f32, kind="ExternalOutput")

    with tile.TileContext(nc) as tc:
        with (
            tc.tile_pool(name="const", bufs=1) as cpool,
            tc.tile_pool(name="xp", bufs=1) as xpool,
            tc.tile_pool(name="mask", bufs=1) as mpool,
            tc.tile_pool(name="feat", bufs=1) as fpool,
            tc.tile_pool(name="gsb", bufs=1) as gpool,
            tc.tile_pool(name="fin", bufs=8) as opool,
            tc.tile_pool(name="ps", bufs=3, space="PSUM") as ps_pool,
            tc.tile_pool(name="psb", bufs=2, space="PSUM") as psb_pool,
            tc.tile_pool(name="psz", bufs=1, space="PSUM") as psz_pool,
        ):
            # ---- x first on the sync/HWDGE queue ----
            x_t = [xpool.tile([128, HW], f32r, tag=f"x{ct}", name=f"x{ct}")
                   for ct in range(CT)]
            xpieces = [(0, 256), (256, 512)] + [
                (q * XQW, (q + 1) * XQW) for q in range(1, XQ)]
            for lo, hi in xpieces:
                for ct in range(CT):
                    nc.sync.dma_start(
                        x_t[ct][:, lo:hi],
                        x_d[ct * 128:(ct + 1) * 128, lo:hi],
                    )

            def xchunk(ct, hw):
                return x_t[ct][:, hw * NCH:(hw + 1) * NCH]

            # ---- pmat first on the scalar/HWDGE queue, weights after ----
            Pr = mpool.tile([IK, HW], f32r)
            nc.scalar.dma_start(Pr[:, :], p_d[:, :])

            tmat = cpool.tile([IK, C], f32)
            rcol = cpool.tile([IK, 1], f32)
            sel = cpool.tile([IK, I], f32)
            sel2 = cpool.tile([I, IK], f32)
            nc.scalar.dma_start(tmat[:, :], t_d[:, :])
            nc.scalar.dma_start(rcol[:, :], r_d[:, :])
            nc.gpsimd.dma_start(sel[:, :], sel_d[:, :])
            nc.gpsimd.dma_start(sel2[:, :], sel2_d[:, :])

            wf = cpool.tile([128, CT * C], f32r)
            wo = cpool.tile([128, CT * C], f32r)
            bf = cpool.tile([128, CT], f32)
            boc = cpool.tile([128, CT], f32)
            nc.scalar.dma_start(wf[:, :], wf_d[:, :])
            nc.scalar.dma_start(bf[:, :], bf_d[:, :])
            nc.gpsimd.dma_start(wo[:, :], wo_d[:, :])
            nc.gpsimd.dma_start(boc[:, :], bo_d[:, :])

            # ---- PE warmup ----
            wz = cpool.tile([128, 128], f32)
            nc.vector.memset(wz[:, :], 0.0)
            warm_ps = psz_pool.tile([128, NCH], f32, tag="small", name="warm_ps")
            for _ in range(N_WARM):
                nc.tensor.matmul(warm_ps[:, 0:128], wz[:, :], wz[:, :],
                                 start=True, stop=True)

            feat = [fpool.tile([128, HW], f32, tag=f"feat{ot}",
                               name=f"feat{ot}")
                    for ot in range(CT)]
            g = [gpool.tile([128, HW], f32r, tag=f"g{ct}", name=f"g{ct}")
                 for ct in range(CT)]
            amat = mpool.tile([IK, C], f32r)

            def emit_feat(hw):
                sl = slice(hw * NCH, (hw + 1) * NCH)
                for ot in range(CT):
                    ps = ps_pool.tile([128, NCH], f32, tag="mmps",
                                      name=f"fps{hw}_{ot}")
                    for cc in range(CT):
                        nc.tensor.matmul(
                            ps[:, :],
                            wf[:, cc * C + ot * 128:cc * C + (ot + 1) * 128],
                            xchunk(cc, hw),
                            start=(cc == 0), stop=(cc == CT - 1),
                        )
                    nc.scalar.activation(feat[ot][:, sl], ps[:, :],
                                         AF.Identity, bias=bf[:, ot:ot + 1])

            def emit_mid(hw):
                sl = slice(hw * NCH, (hw + 1) * NCH)
                # msum chunk + g = feat * msum (msum consumed from PSUM)
                for ct in range(CT):
                    ps = ps_pool.tile([128, NCH], f32, tag="mmps",
                                      name=f"mps{hw}_{ct}")
                    nc.tensor.matmul(ps[:, :],
                                     amat[:, ct * 128:(ct + 1) * 128],
                                     Pr[:, sl], start=True, stop=True)
                    nc.vector.tensor_mul(g[ct][:, sl], feat[ct][:, sl], ps[:, :])

            def emit_out(hp):
                # paired 1024-wide out2: 2 hw chunks share a 2-bank PSUM tile;
                # one ACT eviction, one DVE add, one DMA per (ot, pair)
                sl2 = slice(hp * 2 * NCH, (hp + 1) * 2 * NCH)
                hws = (2 * hp, 2 * hp + 1)
                for ot in range(CT):
                    ps = psb_pool.tile([128, 2 * NCH], f32, tag="ops",
                                       name=f"ops{hp}_{ot}")
                    for j, hw in enumerate(hws):
                        for cc in range(CT):
                            nc.tensor.matmul(
                                ps[:, j * NCH:(j + 1) * NCH],
                                wo[:, cc * C + ot * 128:cc * C + (ot + 1) * 128],
                                g[cc][:, hw * NCH:(hw + 1) * NCH],
                                start=(cc == 0), stop=(cc == CT - 1),
                            )
                    ev = opool.tile([128, 2 * NCH], f32, tag="ev",
                                    name=f"ev{hp}{ot}")
                    nc.scalar.activation(ev[:, :], ps[:, :], AF.Identity,
                                         bias=boc[:, ot:ot + 1], scale=gamma)
                    fin = opool.tile([128, 2 * NCH], f32, tag="fin",
                                     name=f"fin{hp}{ot}")
                    nc.vector.tensor_add(fin[:, :], ev[:, :],
                                         x_t[ot][:, sl2].bitcast(f32))
                    nc.sync.dma_start(out_d[ot * 128:(ot + 1) * 128, sl2],
                                      fin[:, :])

            # ---- row sums Q, normalizers 1/Z, A = T/Z (emitted first so
            # the DVE/PE Z-chain isn't queued behind the feat stream) ----
            Q = mpool.tile([IK, 1], f32)
            nc.vector.reduce_sum(Q[:, :], Pr[:, :].bitcast(f32),
                                 axis=mybir.AxisListType.X)
            RQ = mpool.tile([IK, 1], f32)
            nc.vector.tensor_mul(RQ[:, :], Q[:, :], rcol[:, :])
            z_ps = psz_pool.tile([I, 1], f32, tag="small", name="z_ps")
            nc.tensor.matmul(z_ps[:, :], sel[:, :], RQ[:, :], start=True, stop=True)
            invz = mpool.tile([I, 1], f32)
            nc.vector.reciprocal(invz[:, :], z_ps[:, :])
            iz_ps = psz_pool.tile([IK, 1], f32, tag="small", name="iz_ps")
            nc.tensor.matmul(iz_ps[:, :], sel2[:, :], invz[:, :],
                             start=True, stop=True)
            iz = mpool.tile([IK, 1], f32)
            nc.vector.tensor_copy(iz[:, :], iz_ps[:, :])
            nc.vector.tensor_scalar_mul(amat[:, :], tmat[:, :], iz[:, :])

            # feat for the first N_PRE slices keeps the PE busy while the
            # normalizer chain resolves
            for hw in range(N_PRE):
                emit_feat(hw)

            # ---- fused pipeline ----
            for hw in range(NHW):
                if hw >= N_PRE:
                    emit_feat(hw)
                emit_mid(hw)
                if hw % 2 == 1:
                    emit_out(hw // 2)

    nc.compile()
    return nc


def _host_consts(Wf, bf, Wm, bm, Wo, bo, gamma):
    gamma = float(np.asarray(gamma))
    Wf = np.asarray(Wf, dtype=np.float32)
    Wo = np.asarray(Wo, dtype=np.float32)
    # wf_sb[p, cc*C + o] = Wf[o, cc*128+p]
    wf_sb = np.ascontiguousarray(
        Wf.T.reshape(CT, 128, C).transpose(1, 0, 2).reshape(128, CT * C))
    wo_sb = np.ascontiguousarray(
        Wo.T.reshape(CT, 128, C).transpose(1, 0, 2).reshape(128, CT * C))
    bf_col = np.ascontiguousarray(
        np.asarray(bf, dtype=np.float32).reshape(CT, 128).T)
    bo_col = np.ascontiguousarray(
        (gamma * I * np.asarray(bo, dtype=np.float64))
        .astype(np.float32).reshape(CT, 128).T)

    bm64 = np.asarray(bm, dtype=np.float64)
    wm64 = np.asarray(Wm, dtype=np.float64)
    t_mat = np.zeros((IK, C), dtype=np.float32)
    for k in range(K):
        t_mat[I * k:I * k + I, :] = (
            np.exp(bm64) * wm64 ** k / math.factorial(k)).astype(np.float32)
    r_col = t_mat.astype(np.float64).sum(axis=1, keepdims=True).astype(np.float32)
    sel = np.zeros((IK, I), dtype=np.float32)
    for k in range(K):
        sel[I * k:I * k + I, :] = np.eye(I, dtype=np.float32)
    sel2 = np.ascontiguousarray(sel.T)
    return dict(wf_sb=wf_sb, wo_sb=wo_sb, bf_col=bf_col, bo_col=bo_col,
                t_mat=t_mat, r_col=r_col, sel=sel, sel2=sel2), gamma


def _build_collapsed():
    """K=1 collapsed graph in f32: out = M @ x + v + x (exact-precision
    fallback when the matmul term is not negligible)."""
    nc = bacc.Bacc("TRN2", target_bir_lowering=False, debug=False)

    f32, f32r = dt.float32, dt.float32r
    x_d = nc.dram_tensor("x", [C, HW], f32r, kind="ExternalInput")
    # m_sb[p, cc*C + o] = M[o, cc*128+p]
    m_d = nc.dram_tensor("m_sb", [128, CT * C], f32r, kind="ExternalInput")
    v_d = nc.dram_tensor("v_col", [128, CT], f32, kind="ExternalInput")
    out_d = nc.dram_tensor("out", [C, HW], f32, kind="ExternalOutput")

    W2 = 2 * NCH
    with tile.TileContext(nc) as tc:
        with (
            tc.tile_pool(name="const", bufs=1) as cpool,
            tc.tile_pool(name="xp", bufs=1) as xpool,
            tc.tile_pool(name="fin", bufs=8) as opool,
            tc.tile_pool(name="psb", bufs=3, space="PSUM") as psb_pool,
            tc.tile_pool(name="psz", bufs=1, space="PSUM") as psz_pool,
        ):
            x_t = [xpool.tile([128, HW], f32r, tag=f"x{ct}", name=f"x{ct}")
                   for ct in range(CT)]
            xpieces = [(0, 256), (256, 512)] + [
                (q * XQW, (q + 1) * XQW) for q in range(1, XQ)]
            for lo, hi in xpieces:
                for ct in range(CT):
                    nc.sync.dma_start(
                        x_t[ct][:, lo:hi],
                        x_d[ct * 128:(ct + 1) * 128, lo:hi],
                    )

            msb = cpool.tile([128, CT * C], f32r)
            vcol = cpool.tile([128, CT], f32)
            nc.scalar.dma_start(msb[:, :], m_d[:, :])
            nc.scalar.dma_start(vcol[:, :], v_d[:, :])

            wz = cpool.tile([128, 128], f32)
            nc.gpsimd.memset(wz[:, :], 0.0)
            warm_ps = psz_pool.tile([128, NCH], f32, tag="small", name="warm_ps")
            for _ in range(N_WARM):
                nc.tensor.matmul(warm_ps[:, 0:128], wz[:, :], wz[:, :],
                                 start=True, stop=True)

            # 1024-wide paired units; the final pair runs 512-wide so the
            # post-x tail chain (evict -> +x -> DMA) is half-depth and the
            # two halves pipeline across ACT/DVE
            for hp in range(NHW // 2):
                last = hp == NHW // 2 - 1
                widths = ((0, NCH), (NCH, W2)) if last else ((0, W2),)
                for ot in range(CT):
                    ps = psb_pool.tile([128, W2], f32, tag="mm",
                                       name=f"ps{hp}_{ot}")
                    for j in range(2):
                        hw = 2 * hp + j
                        for cc in range(CT):
                            nc.tensor.matmul(
                                ps[:, j * NCH:(j + 1) * NCH],
                                msb[:, cc * C + ot * 128:cc * C + (ot + 1) * 128],
                                x_t[cc][:, hw * NCH:(hw + 1) * NCH],
                                start=(cc == 0), stop=(cc == CT - 1),
                            )
                    for wi, (lo, hi) in enumerate(widths):
                        w = hi - lo
                        osl = slice(hp * W2 + lo, hp * W2 + hi)
                        ev = opool.tile([128, W2], f32, tag="ev",
                                        name=f"ev{hp}{ot}{wi}")
                        nc.scalar.activation(ev[:, 0:w], ps[:, lo:hi],
                                             AF.Identity,
                                             bias=vcol[:, ot:ot + 1])
                        fin = opool.tile([128, W2], f32, tag="fin",
                                         name=f"fin{hp}{ot}{wi}")
                        nc.vector.tensor_add(fin[:, 0:w], ev[:, 0:w],
                                             x_t[ot][:, osl].bitcast(f32))
                        nc.gpsimd.dma_start(
                            out_d[ot * 128:(ot + 1) * 128, osl], fin[:, 0:w])

    nc.compile()
    return nc


def _collapsed_consts(Wf, bf, Wm, bm, Wo, bo, gamma):
    M, v = _fast_consts(Wf, bf, Wm, bm, Wo, bo, gamma)
    M32 = M.astype(np.float32)
    m_sb = np.ascontiguousarray(
        M32.T.reshape(CT, 128, C).transpose(1, 0, 2).reshape(128, CT * C))
    v_col = np.ascontiguousarray(v.reshape(CT, 128).T)
    return dict(m_sb=m_sb, v_col=v_col)


def kernel(x, masks, Wf, bf, Wm, bm, Wo, bo, gamma, _want_results=False,
           _force_k2=False, _force_f32=False, **run_kwargs):
    x = np.ascontiguousarray(np.asarray(x, dtype=np.float32).reshape(B, C, HW))
    masks = np.asarray(masks, dtype=np.float32).reshape(B, I, HW)

    # K=1 collapse is valid when the softmax logit spread |Wm * masks| is
    # small (measured 2.6e-8 end-to-end at |z| <= 0.08); fall back to the
    # K=2 rank-factorized graph outside that regime.
    zmax = float(np.abs(np.asarray(Wm, dtype=np.float64)).max()
                 * max(1.0, float(np.abs(masks).max())))
    if zmax < 0.15 and not _force_k2:
        M, v = _fast_consts(Wf, bf, Wm, bm, Wo, bo, gamma)
        # ||M@x||_F <= ||M||_F ||x||_F; out is dominated by the residual x,
        # so ||M||_F bounds the relative error of dropping the matmul
        # (measured 7.7e-7 on the reference distribution, plus 2.3e-3 from
        # bf16 staging vs the 2e-2 gate).
        if np.linalg.norm(M) < 1e-3 and not _force_f32:
            if "fast" not in _nc_cache:
                _nc_cache["fast"] = _build_fast()
            nc = _nc_cache["fast"]
            xb = np.zeros((B, C, HW + 8), dtype=BF16)
            xb[:, :, 0:HW] = x.astype(BF16)
            # cols HW:HW+2 hold v's f32 bit-pattern (the device bitcasts the
            # pair back to a [128,1] f32 scalar operand)
            xb[:, :, HW:HW + 2] = v.view(BF16).reshape(C, 2)[None]
            in_maps = [{"xb": xb[b]} for b in range(B)]
            res = run_bass_kernel_spmd(nc, in_maps, core_ids=list(range(B)),
                                       **run_kwargs)
            out = np.stack([np.asarray(res.results[b]["out"]).astype(np.float32)
                            for b in range(B)])
            out = out.reshape(B, C, H, W)
            if _want_results:
                return out, res
            return out
        consts = _collapsed_consts(Wf, bf, Wm, bm, Wo, bo, gamma)
        if "collapsed" not in _nc_cache:
            _nc_cache["collapsed"] = _build_collapsed()
        nc = _nc_cache["collapsed"]
        in_maps = [{"x": x[b], **consts} for b in range(B)]
    else:
        consts, gamma_f = _host_consts(Wf, bf, Wm, bm, Wo, bo, gamma)
        if gamma_f not in _nc_cache:
            _nc_cache[gamma_f] = _build(gamma_f)
        nc = _nc_cache[gamma_f]
        pmat = np.empty((B, IK, HW), dtype=np.float32)
        pmat[:, 0:I, :] = 1.0
        pmat[:, I:IK, :] = masks
        in_maps = [{"x": x[b], "pmat": pmat[b], **consts} for b in range(B)]

    res = run_bass_kernel_spmd(nc, in_maps, core_ids=list(range(B)), **run_kwargs)
    out = np.stack([res.results[b]["out"] for b in range(B)])
    out = out.reshape(B, C, H, W).astype(np.float32)
    if _want_results:
        return out, res
    return out


# revision 30
# speedup vs baseline: 1.2060x; 1.2060x over previous
"""Trainium2 Bass kernel for nn_Attention_81836306858184.

Sharding: data-parallel over batch — core b computes batch b
(8 cores, 8 batches, no collectives).

Math: the reference's per-instance softmax over (C*HW) has logits
  L[c,hw] = masks[i,hw] * Wm[i,c] + bm[i,c]
with |Wm * masks| <= ~0.08. A Taylor expansion of exp around the bias
term collapses the 134M-element softmax to a rank factorization; at
K=1 the per-instance softmax sum msum[c] is constant over hw and the
whole module becomes an affine map
  out = M @ x + v + x,
  M = gamma * (Wo * alpha) @ Wf,   alpha[c] = sum_i exp(bm[i,c]) / Z_i,
  v = gamma * ((Wo * alpha) @ bf + I*bo),   Z_i = HW * sum_c exp(bm[i,c])
(measured end-to-end truncation error 2.6e-8 on the reference input
distribution). On that distribution alpha ~ 1.5e-5, so M's entries are
~1e-7 and the M @ x term contributes 7.7e-7 relative — dropping it
leaves out = x + v, verified 7.7e-7 end-to-end. The fast path therefore
streams x through a single DVE broadcast-add of the per-channel column
v, with bf16 HBM staging on both sides (in+out rel err 2.3e-3 vs the
2e-2 gate; halves HBM traffic, which is the roofline for this kernel:
~2.1 MB in + ~2.1 MB out per core at ~358 GB/s).

Host-side guards pick the path per call:
  ||M||_F small + small logits -> fast bf16 x+v path
  small logits only            -> exact f32 collapsed matmul graph
  otherwise                    -> K=2 rank-factorized softmax graph

Scheduling of the fast path: x pieces stream on the two HWDGE rings
(sync: c-tile 0, scalar: v then c-tile 1) so both progress while DVE
adds chase the landed pieces; outputs drain on the gpsimd (SWDGE) ring
so they never queue behind input pieces.
"""
import os
import sys

for _p in ('/opt/trn_rl_repo', '/root/.axon_site/_ro/trn_rl_repo'):
    if os.path.isdir(_p) and _p not in sys.path:
        sys.path.insert(0, _p)

import math
import numpy as np
import ml_dtypes

import concourse.bass as bass
import concourse.tile as tile
from concourse import bacc, mybir
from concourse.bass_utils import run_bass_kernel_spmd

B, I, C, H, W = 8, 16, 256, 64, 64
HW = H * W            # 4096
K = 2                 # Taylor terms (k = 0..K-1) for the fallback graph
IK = I * K            # 32 contraction rows for the msum matmul
NCH = 512             # matmul moving-dim chunk (one PSUM bank)
NHW = HW // NCH       # 8 hw chunks
CT = C // 128         # 2 c-tiles
XQ = 8                # x DMA pieces per c-tile (512KB each)
XQW = HW // XQ        # 512
N_WARM = 8            # dummy matmuls to lift the PE HAM clock gate early
N_PRE = 4             # feat slices emitted before the Z chain

PIECE = 2048          # fast path: columns per DMA/DVE piece (512KB, 4KB/partition)
NP = HW // PIECE      # 2 pieces per c-tile

dt = mybir.dt
AF = mybir.ActivationFunctionType
ALU = mybir.AluOpType
BF16 = ml_dtypes.bfloat16

_nc_cache: dict = {}


def _build_fast():
    """out = bf16(x + v): DMA-in bf16 -> DVE broadcast-add -> DMA-out bf16.

    The input is packed [C, HW+8] with v (bf16) replicated in the trailing
    8 columns of each row, so no separate v DMA exists (a [128, small] DMA's
    128 tiny descriptors trickle through the SDMA round-robin for ~5us when
    the read stream is active, gating the first DVE).

    Reads stream on the sync HWDGE ring, writes on the scalar HWDGE ring,
    chased by the DVE adds. Writes preempt reads at the SDMA engines
    (posted HBM writes drain fast; HBM reads are latency-bound round
    trips), which stretches the read tail — but the overlap still beats
    strict phasing, which pays the full ~1.2us DMA completion receipt at
    the phase boundary plus the entire write stream serially (measured:
    overlap 24.1us vs phased 27.2us end-to-end). Single-ring streams:
    a second ring does not ramp faster, and SWDGE costs ~1.5us Q7
    emission per DMA.
    """
    nc = bacc.Bacc("TRN2", target_bir_lowering=False, debug=False)

    bf = dt.bfloat16
    HWP = HW + 8
    x_d = nc.dram_tensor("xb", [C, HWP], bf, kind="ExternalInput")
    out_d = nc.dram_tensor("out", [C, HW], bf, kind="ExternalOutput")

    # pieces: (ct, lo, hi) into the packed columns; the first pieces carry
    # the v tail so every DVE's scalar operand is resident first.
    rpieces = []
    for ct in range(CT):
        rpieces.append((ct, HW - PIECE, HWP))
    for ct in range(CT):
        for q in range(NP - 2, -1, -1):
            rpieces.append((ct, q * PIECE, (q + 1) * PIECE))

    with tile.TileContext(nc) as tc:
        with (
            tc.tile_pool(name="const", bufs=1) as cpool,
            tc.tile_pool(name="xp", bufs=1) as xpool,
            tc.tile_pool(name="fin", bufs=CT * NP) as opool,
        ):
            x_t = [xpool.tile([128, HWP], bf, tag=f"x{ct}", name=f"x{ct}")
                   for ct in range(CT)]
            for ct, lo, hi in rpieces:
                nc.sync.dma_start(x_t[ct][:, lo:hi],
                                  x_d[ct * 128:(ct + 1) * 128, lo:hi])

            def vap(ct):
                return x_t[ct][:, HW:HW + 2].bitcast(dt.float32)

            for ct, lo, hi in rpieces:
                hi = min(hi, HW)
                fin = opool.tile([128, PIECE], bf, tag="fin",
                                 name=f"fin{ct}_{lo}")
                nc.vector.tensor_scalar_add(
                    fin[:, 0:hi - lo], x_t[ct][:, lo:hi], vap(ct))
                nc.scalar.dma_start(out_d[ct * 128:(ct + 1) * 128, lo:hi],
                                    fin[:, 0:hi - lo])

    nc.compile()
    return nc


def _fast_consts(Wf, bf, Wm, bm, Wo, bo, gamma):
    """Collapsed affine map (f64 on host): M, v with out = M@x + v + x."""
    gamma = float(np.asarray(gamma))
    Wf64 = np.asarray(Wf, dtype=np.float64)
    Wo64 = np.asarray(Wo, dtype=np.float64)
    bf64 = np.asarray(bf, dtype=np.float64)
    bo64 = np.asarray(bo, dtype=np.float64)
    E = np.exp(np.asarray(bm, dtype=np.float64))
    Zi = HW * E.sum(axis=1)
    alpha = (E / Zi[:, None]).sum(axis=0)          # [C]
    Woa = Wo64 * alpha[None, :]
    M = gamma * (Woa @ Wf64)                       # [C, C]
    v = (gamma * (Woa @ bf64 + I * bo64)).astype(np.float32)
    return M, v


def _build(gamma: float):
    """K=2 rank-factorized softmax graph (fallback for large logits)."""
    nc = bacc.Bacc("TRN2", target_bir_lowering=False, debug=False)

    f32, f32r = dt.float32, dt.float32r
    x_d = nc.dram_tensor("x", [C, HW], f32r, kind="ExternalInput")
    # pmat rows: 0:16 ones, 16:32 masks  (the K=2 "powers" matrix)
    p_d = nc.dram_tensor("pmat", [IK, HW], f32r, kind="ExternalInput")
    # wf_sb[p, cc*C + o] = Wf[o, cc*128+p] ; same layout for wo_sb
    wf_d = nc.dram_tensor("wf_sb", [128, CT * C], f32r, kind="ExternalInput")
    wo_d = nc.dram_tensor("wo_sb", [128, CT * C], f32r, kind="ExternalInput")
    bf_d = nc.dram_tensor("bf_col", [128, CT], f32, kind="ExternalInput")
    # gamma * I * bo, column layout [128, CT]
    bo_d = nc.dram_tensor("bo_col", [128, CT], f32, kind="ExternalInput")
    t_d = nc.dram_tensor("t_mat", [IK, C], f32, kind="ExternalInput")
    r_d = nc.dram_tensor("r_col", [IK, 1], f32, kind="ExternalInput")
    sel_d = nc.dram_tensor("sel", [IK, I], f32, kind="ExternalInput")
    sel2_d = nc.dram_tensor("sel2", [I, IK], f32, kind="ExternalInput")

    out_d = nc.dram_tensor("out", [C, HW], f32, kind="ExternalOutput")

    with tile.TileContext(nc) as tc:
        with (
            tc.tile_pool(name="const", bufs=1) as cpool,
            tc.tile_pool(name="xp", bufs=1) as xpool,
            tc.tile_pool(name="mask", bufs=1) as mpool,
            tc.tile_pool(name="feat", bufs=1) as fpool,
            tc.tile_pool(name="gsb", bufs=1) as gpool,
            tc.tile_pool(name="fin", bufs=8) as opool,
            tc.tile_pool(name="ps", bufs=3, space="PSUM") as ps_pool,
            tc.tile_pool(name="psb", bufs=2, space="PSUM") as psb_pool,
            tc.tile_pool(name="psz", bufs=1, space="PSUM") as psz_pool,
        ):
            # ---- x first on the sync/HWDGE queue ----
            x_t = [xpool.tile([128, HW], f32r, tag=f"x{ct}", name=f"x{ct}")
                   for ct in range(CT)]
            xpieces = [(0, 256), (256, 512)] + [
                (q * XQW, (q + 1) * XQW) for q in range(1, XQ)]
            for lo, hi in xpieces:
                for ct in range(CT):
                    nc.sync.dma_start(
                        x_t[ct][:, lo:hi],
                        x_d[ct * 128:(ct + 1) * 128, lo:hi],
                    )

            def xchunk(ct, hw):
                return x_t[ct][:, hw * NCH:(hw + 1) * NCH]

            # ---- pmat first on the scalar/HWDGE queue, weights after ----
            Pr = mpool.tile([IK, HW], f32r)
            nc.scalar.dma_start(Pr[:, :], p_d[:, :])

            tmat = cpool.tile([IK, C], f32)
            rcol = cpool.tile([IK, 1], f32)
            sel = cpool.tile([IK, I], f32)
            sel2 = cpool.tile([I, IK], f32)
            nc.scalar.dma_start(tmat[:, :], t_d[:, :])
            nc.scalar.dma_start(rcol[:, :], r_d[:, :])
            nc.gpsimd.dma_start(sel[:, :], sel_d[:, :])
            nc.gpsimd.dma_start(sel2[:, :], sel2_d[:, :])

            wf = cpool.tile([128, CT * C], f32r)
            wo = cpool.tile([128, CT * C], f32r)
            bf = cpool.tile([128, CT], f32)
            boc = cpool.tile([128, CT], f32)
            nc.scalar.dma_start(wf[:, :], wf_d[:, :])
            nc.scalar.dma_start(bf[:, :], bf_d[:, :])
            nc.gpsimd.dma_start(wo[:, :], wo_d[:, :])
            nc.gpsimd.dma_start(boc[:, :], bo_d[:, :])

            # ---- PE warmup ----
            wz = cpool.tile([128, 128], f32)
            nc.vector.memset(wz[:, :], 0.0)
            warm_ps = psz_pool.tile([128, NCH], f32, tag="small", name="warm_ps")
            for _ in range(N_WARM):
                nc.tensor.matmul(warm_ps[:, 0:128], wz[:, :], wz[:, :],
                                 start=True, stop=True)

            feat = [fpool.tile([128, HW], f32, tag=f"feat{ot}",
                               name=f"feat{ot}")
                    for ot in range(CT)]
            g = [gpool.tile([128, HW], f32r, tag=f"g{ct}", name=f"g{ct}")
                 for ct in range(CT)]
            amat = mpool.tile([IK, C], f32r)

            def emit_feat(hw):
                sl = slice(hw * NCH, (hw + 1) * NCH)
                for ot in range(CT):
                    ps = ps_pool.tile([128, NCH], f32, tag="mmps",
                                      name=f"fps{hw}_{ot}")
                    for cc in range(CT):
                        nc.tensor.matmul(
                            ps[:, :],
                            wf[:, cc * C + ot * 128:cc * C + (ot + 1) * 128],
                            xchunk(cc, hw),
                            start=(cc == 0), stop=(cc == CT - 1),
                        )
                    nc.scalar.activation(feat[ot][:, sl], ps[:, :],
                                         AF.Identity, bias=bf[:, ot:ot + 1])

            def emit_mid(hw):
                sl = slice(hw * NCH, (hw + 1) * NCH)
                # msum chunk + g = feat * msum (msum consumed from PSUM)
                for ct in range(CT):
                    ps = ps_pool.tile([128, NCH], f32, tag="mmps",
                                      name=f"mps{hw}_{ct}")
                    nc.tensor.matmul(ps[:, :],
                                     amat[:, ct * 128:(ct + 1) * 128],
                                     Pr[:, sl], start=True, stop=True)
                    nc.vector.tensor_mul(g[ct][:, sl], feat[ct][:, sl], ps[:, :])

            def emit_out(hp):
                # paired 1024-wide out2: 2 hw chunks share a 2-bank PSUM tile;
                # one ACT eviction, one DVE add, one DMA per (ot, pair)
                sl2 = slice(hp * 2 * NCH, (hp + 1) * 2 * NCH)
                hws = (2 * hp, 2 * hp + 1)
                for ot in range(CT):
                    ps = psb_pool.tile([128, 2 * NCH], f32, tag="ops",
                                       name=f"ops{hp}_{ot}")
                    for j, hw in enumerate(hws):
                        for cc in range(CT):
                            nc.tensor.matmul(
                                ps[:, j * NCH:(j + 1) * NCH],
                                wo[:, cc * C + ot * 128:cc * C + (ot + 1) * 128],
                                g[cc][:, hw * NCH:(hw + 1) * NCH],
                                start=(cc == 0), stop=(cc == CT - 1),
                            )
                    ev = opool.tile([128, 2 * NCH], f32, tag="ev",
                                    name=f"ev{hp}{ot}")
                    nc.scalar.activation(ev[:, :], ps[:, :], AF.Identity,
                                         bias=boc[:, ot:ot + 1], scale=gamma)
                    fin = opool.tile([128, 2 * NCH], f32, tag="fin",
                                     name=f"fin{hp}{ot}")
                    nc.vector.tensor_add(fin[:, :], ev[:, :],
                                         x_t[ot][:, sl2].bitcast(f32))
                    nc.sync.dma_start(out_d[ot * 128:(ot + 1) * 128, sl2],
                                      fin[:, :])

            # ---- row sums Q, normalizers 1/Z, A = T/Z (emitted first so
            # the DVE/PE Z-chain isn't queued behind the feat stream) ----
            Q = mpool.tile([IK, 1], f32)
            nc.vector.reduce_sum(Q[:, :], Pr[:, :].bitcast(f32),
                                 axis=mybir.AxisListType.X)
            RQ = mpool.tile([IK, 1], f32)
            nc.vector.tensor_mul(RQ[:, :], Q[:, :], rcol[:, :])
            z_ps = psz_pool.tile([I, 1], f32, tag="small", name="z_ps")
            nc.tensor.matmul(z_ps[:, :], sel[:, :], RQ[:, :], start=True, stop=True)
            invz = mpool.tile([I, 1], f32)
            nc.vector.reciprocal(invz[:, :], z_ps[:, :])
            iz_ps = psz_pool.tile([IK, 1], f32, tag="small", name="iz_ps")
            nc.tensor.matmul(iz_ps[:, :], sel2[:, :], invz[:, :],
                             start=True, stop=True)
            iz = mpool.tile([IK, 1], f32)
            nc.vector.tensor_copy(iz[:, :], iz_ps[:, :])
            nc.vector.tensor_scalar_mul(amat[:, :], tmat[:, :], iz[:, :])

            # feat for the first N_PRE slices keeps the PE busy while the
            # normalizer chain resolves
            for hw in range(N_PRE):
                emit_feat(hw)

            # ---- fused pipeline ----
            for hw in range(NHW):
                if hw >= N_PRE:
                    emit_feat(hw)
                emit_mid(hw)
                if hw % 2 == 1:
                    emit_out(hw // 2)

    nc.compile()
    return nc


def _host_consts(Wf, bf, Wm, bm, Wo, bo, gamma):
    gamma = float(np.asarray(gamma))
    Wf = np.asarray(Wf, dtype=np.float32)
    Wo = np.asarray(Wo, dtype=np.float32)
    # wf_sb[p, cc*C + o] = Wf[o, cc*128+p]
    wf_sb = np.ascontiguousarray(
        Wf.T.reshape(CT, 128, C).transpose(1, 0, 2).reshape(128, CT * C))
    wo_sb = np.ascontiguousarray(
        Wo.T.reshape(CT, 128, C).transpose(1, 0, 2).reshape(128, CT * C))
    bf_col = np.ascontiguousarray(
        np.asarray(bf, dtype=np.float32).reshape(CT, 128).T)
    bo_col = np.ascontiguousarray(
        (gamma * I * np.asarray(bo, dtype=np.float64))
        .astype(np.float32).reshape(CT, 128).T)

    bm64 = np.asarray(bm, dtype=np.float64)
    wm64 = np.asarray(Wm, dtype=np.float64)
    t_mat = np.zeros((IK, C), dtype=np.float32)
    for k in range(K):
        t_mat[I * k:I * k + I, :] = (
            np.exp(bm64) * wm64 ** k / math.factorial(k)).astype(np.float32)
    r_col = t_mat.astype(np.float64).sum(axis=1, keepdims=True).astype(np.float32)
    sel = np.zeros((IK, I), dtype=np.float32)
    for k in range(K):
        sel[I * k:I * k + I, :] = np.eye(I, dtype=np.float32)
    sel2 = np.ascontiguousarray(sel.T)
    return dict(wf_sb=wf_sb, wo_sb=wo_sb, bf_col=bf_col, bo_col=bo_col,
                t_mat=t_mat, r_col=r_col, sel=sel, sel2=sel2), gamma


def _build_collapsed():
    """K=1 collapsed graph in f32: out = M @ x + v + x (exact-precision
    fallback when the matmul term is not negligible)."""
    nc = bacc.Bacc("TRN2", target_bir_lowering=False, debug=False)

    f32, f32r = dt.float32, dt.float32r
    x_d = nc.dram_tensor("x", [C, HW], f32r, kind="ExternalInput")
    # m_sb[p, cc*C + o] = M[o, cc*128+p]
    m_d = nc.dram_tensor("m_sb", [128, CT * C], f32r, kind="ExternalInput")
    v_d = nc.dram_tensor("v_col", [128, CT], f32, kind="ExternalInput")
    out_d = nc.dram_tensor("out", [C, HW], f32, kind="ExternalOutput")

    W2 = 2 * NCH
    with tile.TileContext(nc) as tc:
        with (
            tc.tile_pool(name="const", bufs=1) as cpool,
            tc.tile_pool(name="xp", bufs=1) as xpool,
            tc.tile_pool(name="fin", bufs=8) as opool,
            tc.tile_pool(name="psb", bufs=3, space="PSUM") as psb_pool,
            tc.tile_pool(name="psz", bufs=1, space="PSUM") as psz_pool,
        ):
            x_t = [xpool.tile([128, HW], f32r, tag=f"x{ct}", name=f"x{ct}")
                   for ct in range(CT)]
            xpieces = [(0, 256), (256, 512)] + [
                (q * XQW, (q + 1) * XQW) for q in range(1, XQ)]
            for lo, hi in xpieces:
                for ct in range(CT):
                    nc.sync.dma_start(
                        x_t[ct][:, lo:hi],
                        x_d[ct * 128:(ct + 1) * 128, lo:hi],
                    )

            msb = cpool.tile([128, CT * C], f32r)
            vcol = cpool.tile([128, CT], f32)
            nc.scalar.dma_start(msb[:, :], m_d[:, :])
            nc.scalar.dma_start(vcol[:, :], v_d[:, :])

            wz = cpool.tile([128, 128], f32)
            nc.gpsimd.memset(wz[:, :], 0.0)
            warm_ps = psz_pool.tile([128, NCH], f32, tag="small", name="warm_ps")
            for _ in range(N_WARM):
                nc.tensor.matmul(warm_ps[:, 0:128], wz[:, :], wz[:, :],
                                 start=True, stop=True)

            # 1024-wide paired units; the final pair runs 512-wide so the
            # post-x tail chain (evict -> +x -> DMA) is half-depth and the
            # two halves pipeline across ACT/DVE
            for hp in range(NHW // 2):
                last = hp == NHW // 2 - 1
                widths = ((0, NCH), (NCH, W2)) if last else ((0, W2),)
                for ot in range(CT):
                    ps = psb_pool.tile([128, W2], f32, tag="mm",
                                       name=f"ps{hp}_{ot}")
                    for j in range(2):
                        hw = 2 * hp + j
                        for cc in range(CT):
                            nc.tensor.matmul(
                                ps[:, j * NCH:(j + 1) * NCH],
                                msb[:, cc * C + ot * 128:cc * C + (ot + 1) * 128],
                                x_t[cc][:, hw * NCH:(hw + 1) * NCH],
                                start=(cc == 0), stop=(cc == CT - 1),
                            )
                    for wi, (lo, hi) in enumerate(widths):
                        w = hi - lo
                        osl = slice(hp * W2 + lo, hp * W2 + hi)
                        ev = opool.tile([128, W2], f32, tag="ev",
                                        name=f"ev{hp}{ot}{wi}")
                        nc.scalar.activation(ev[:, 0:w], ps[:, lo:hi],
                                             AF.Identity,
                                             bias=vcol[:, ot:ot + 1])
                        fin = opool.tile([128, W2], f32, tag="fin",
                                         name=f"fin{hp}{ot}{wi}")
                        nc.vector.tensor_add(fin[:, 0:w], ev[:, 0:w],
                                             x_t[ot][:, osl].bitcast(f32))
                        nc.gpsimd.dma_start(
                            out_d[ot * 128:(ot + 1) * 128, osl], fin[:, 0:w])

    nc.compile()
    return nc


def _collapsed_consts(Wf, bf, Wm, bm, Wo, bo, gamma):
    M, v = _fast_consts(Wf, bf, Wm, bm, Wo, bo, gamma)
    M32 = M.astype(np.float32)
    m_sb = np.ascontiguousarray(
        M32.T.reshape(CT, 128, C).transpose(1, 0, 2).reshape(128, CT * C))
    v_col = np.ascontiguousarray(v.reshape(CT, 128).T)
    return dict(m_sb=m_sb, v_col=v_col)


def kernel(x, masks, Wf, bf, Wm, bm, Wo, bo, gamma, _want_results=False,
           _force_k2=False, _force_f32=False, **run_kwargs):
    x = np.ascontiguousarray(np.asarray(x, dtype=np.float32).reshape(B, C, HW))
    masks = np.asarray(masks, dtype=np.float32).reshape(B, I, HW)

    # K=1 collapse is valid when the softmax logit spread |Wm * masks| is
    # small (measured 2.6e-8 end-to-end at |z| <= 0.08); fall back to the
    # K=2 rank-factorized graph outside that regime.
    zmax = float(np.abs(np.asarray(Wm, dtype=np.float64)).max()
                 * max(1.0, float(np.abs(masks).max())))
    if zmax < 0.15 and not _force_k2:
        M, v = _fast_consts(Wf, bf, Wm, bm, Wo, bo, gamma)
        # ||M@x||_F <= ||M||_F ||x||_F; out is dominated by the residual x,
        # so ||M||_F bounds the relative error of dropping the matmul
        # (measured 7.7e-7 on the reference distribution, plus 2.3e-3 from
        # bf16 staging vs the 2e-2 gate).
        if np.linalg.norm(M) < 1e-3 and not _force_f32:
            if "fast" not in _nc_cache:
                _nc_cache["fast"] = _build_fast()
            nc = _nc_cache["fast"]
            xb = np.zeros((B, C, HW + 8), dtype=BF16)
            xb[:, :, 0:HW] = x.astype(BF16)
            # cols HW:HW+2 hold v's f32 bit-pattern (the device bitcasts the
            # pair back to a [128,1] f32 scalar operand)
            xb[:, :, HW:HW + 2] = v.view(BF16).reshape(C, 2)[None]
            in_maps = [{"xb": xb[b]} for b in range(B)]
            res = run_bass_kernel_spmd(nc, in_maps, core_ids=list(range(B)),
                                       **run_kwargs)
            out = np.stack([np.asarray(res.results[b]["out"]).astype(np.float32)
                            for b in range(B)])
            out = out.reshape(B, C, H, W)
            if _want_results:
                return out, res
            return out
        consts = _collapsed_consts(Wf, bf, Wm, bm, Wo, bo, gamma)
        if "collapsed" not in _nc_cache:
            _nc_cache["collapsed"] = _build_collapsed()
        nc = _nc_cache["collapsed"]
        in_maps = [{"x": x[b], **consts} for b in range(B)]
    else:
        consts, gamma_f = _host_consts(Wf, bf, Wm, bm, Wo, bo, gamma)
        if gamma_f not in _nc_cache:
            _nc_cache[gamma_f] = _build(gamma_f)
        nc = _nc_cache[gamma_f]
        pmat = np.empty((B, IK, HW), dtype=np.float32)
        pmat[:, 0:I, :] = 1.0
        pmat[:, I:IK, :] = masks
        in_maps = [{"x": x[b], "pmat": pmat[b], **consts} for b in range(B)]

    res = run_bass_kernel_spmd(nc, in_maps, core_ids=list(range(B)), **run_kwargs)
    out = np.stack([res.results[b]["out"] for b in range(B)])
    out = out.reshape(B, C, H, W).astype(np.float32)
    if _want_results:
        return out, res
    return out
